# revision 32
# baseline (speedup 1.0000x reference)
"""Trainium2 Bass kernel for a transformer block (dense_transformer).

Reference computation (B=2, N=2048, C=1024, 16 heads, head_dim=64):
    x = x + attn(LN1(x))            # attn WITHOUT output projection; heads
                                    # interleaved by the faithful reshape
    out = x + MLP(LN2(x))           # MLP = relu(x@w1+b1)@w2+b2

Sharding: 8 cores; core c handles batch b=c//4 and heads 4g..4g+3 (g=c%4).
Because the reference reshapes [B,H,N,hd]->[B,N,C] without transposing
heads back, head h's attention output occupies output rows
[128h, 128h+128) of batch b: out[b, 128h+r, 64m+d] = attn_h[16r+m, d].
So a (batch, 4-head) shard produces a contiguous [512, 1024] output slab
and the whole residual+MLP for those rows is local to the core.

v2 design (evidence-driven from the v1 trace):
  * LN gamma/beta folded into the consuming weights on the host
    (W' = g[:,None]*W, b' = b + beta@W) - exact; LN on chip is the pure
    (x-mu)*rsqrt(var+eps), with rsqrt = exp(-0.5*ln(var+eps)) so every
    ACT function used (ln/exp/relu/copy/identity) lives in ONE
    activation-table set -> zero table reloads.
  * x chunks DMA first; LN1(DVE) / transposes(PE) / QKV(PE) emitted
    interleaved per block so the PE stream never drains.
  * Attention is n-blocked in two 1024-wide passes per head; scoresT
    psum [128m,1024n] -> exp on ACT (sole exp engine = the attention
    bottleneck, so it does nothing else).  AV computed transposed
    (oT[d,n] = sum_m v[m,d] exp[m,n]) as 512-wide full-rate streams;
    ones column in v gives softmax denominators in oT row 64.  The
    interleaved output layout is restored by 16 stride-16 PE transposes
    per head; normalization and the residual add are fused into one DVE
    scalar_tensor_tensor per 64-col group.
  * MLP weights stream in big-contiguous-run tiles: w1 during
    attention, w2 racing MLP1/2 on the gpsimd DMA queue; prefetch issue
    points are placed after the previous slot-user's last read.
"""

import os
import sys
from contextlib import ExitStack

for _p in ("/opt/trn_rl_repo", "/root/.axon_site/_ro/trn_rl_repo"):
    if os.path.isdir(_p) and _p not in sys.path:
        sys.path.insert(0, _p)

import numpy as np

import concourse.bass as bass
import concourse.tile as tile
from concourse import bacc, mybir
from concourse.bass_utils import run_bass_kernel_spmd
from concourse.masks import make_identity

F32 = mybir.dt.float32
F32R = mybir.dt.float32r
BF16 = mybir.dt.bfloat16
AF = mybir.ActivationFunctionType
OP = mybir.AluOpType

P = 128
B, N, C = 2, 2048, 1024
H, HD = 16, 64
H4 = 4 * C
EPS = 1e-5
SCALE = 1.0 / 32.0  # 1/sqrt(C)

NH = 4            # heads per core
NPAIR = 2         # head pairs per core
ROWS = NH * P     # output rows per core (512)
NCHUNK = N // P   # 16 sequence chunks
CCH = C // P      # 8 channel chunks
HKN = H4 // P     # 32 hidden chunks

_TS = bass.ts


def _emit(nc):
    x = nc.dram_tensor("x", (N, C), F32, kind="ExternalInput").ap()
    xown = nc.dram_tensor("xown", (ROWS, C), F32, kind="ExternalInput").ap()
    wqkv = nc.dram_tensor("wqkv", (C, 3 * NH * HD), BF16,
                          kind="ExternalInput").ap()
    qb = nc.dram_tensor("qb", (NH * HD,), F32, kind="ExternalInput").ap()
    kb = nc.dram_tensor("kb", (NH * HD,), F32, kind="ExternalInput").ap()
    vb = nc.dram_tensor("vb", (NH * HD,), F32, kind="ExternalInput").ap()
    w1 = nc.dram_tensor("w1", (C, H4), BF16, kind="ExternalInput").ap()
    b1 = nc.dram_tensor("b1", (H4,), F32, kind="ExternalInput").ap()
    w2 = nc.dram_tensor("w2", (H4, C), BF16, kind="ExternalInput").ap()
    b2 = nc.dram_tensor("b2", (C,), F32R, kind="ExternalInput").ap()
    y = nc.dram_tensor("y", (ROWS, C), F32, kind="ExternalOutput").ap()

    reps = int(os.environ.get("KERNEL_REPS", "1"))
    with tile.TileContext(nc) as tc:
        for _ in range(reps):
            _body(tc, nc, x, xown, wqkv, qb, kb, vb, w1, b1, w2, b2, y)
    return nc


def _body(tc, nc, x, xown, wqkv, qb, kb, vb, w1, b1, w2, b2, y):
    w1r = w1.rearrange("(k p) hh -> p k hh", p=P)
    w2r = w2.rearrange("(k p) c -> p k c", p=P)
    with ExitStack() as ctx:
        singles = ctx.enter_context(tc.tile_pool(name="singles", bufs=1))
        w1p = ctx.enter_context(tc.tile_pool(name="w1p", bufs=4))
        w1_tiles = []

        id_b = singles.tile([P, P], BF16)
        make_identity(nc, id_b[:])
        eps_t = singles.tile([P, 1], F32)
        nc.vector.memset(eps_t[:], EPS)
        ones_f = singles.tile([1, P], F32)
        nc.vector.memset(ones_f[:], 1.0)
        ones_row = singles.tile([1, P], F32R)
        nc.vector.tensor_copy(ones_row[:], ones_f[:])
        b1_sb = singles.tile([P, HKN], F32)
        b2_sb = singles.tile([1, C], F32R)
        qb_sb = singles.tile([P, NPAIR], F32)
        kb_sb = singles.tile([P, NPAIR], F32)
        vb_sb = singles.tile([P, NPAIR], F32)

        x2 = singles.tile([P, NH, C], F32)
        ln2T = singles.tile([P, CCH, ROWS], BF16)

        with ExitStack() as actx:
            apool = actx.enter_context(tc.tile_pool(name="apool", bufs=1))
            xk = apool.tile([P, NH, C], F32)
            qT = apool.tile([P, NPAIR, N], BF16)
            kT = apool.tile([P, NPAIR, N], BF16)
            vT = apool.tile([P, NPAIR, N], BF16)
            v_sb = apool.tile([P, NH, NCHUNK, HD + 1], BF16)
            nc.vector.memset(v_sb[:, :, :, HD:HD + 1], 1.0)
            oT_sb = apool.tile([P, 4, 512], BF16)   # rows 0..64 used
            denT = apool.tile([P, NH, NCHUNK], F32)
            rdenT = apool.tile([P, NH, NCHUNK], F32)

            # ---------- phase 1: LN1 + transpose + QKV ----------------
            with (
                tc.tile_pool(name="ph1", bufs=3) as ph1,
                tc.tile_pool(name="ph1b", bufs=1) as ph1b,
                tc.tile_pool(name="xpool", bufs=6) as xpool,
                tc.tile_pool(name="pp_q", bufs=3, space="PSUM") as pp_q,
                tc.tile_pool(name="pp_t1", bufs=2, space="PSUM") as pp_t1,
            ):
                ln1T = ph1b.tile([P, CCH, N], BF16)
                wqkv_sb = ph1b.tile([P, CCH, 3 * NH * HD], BF16)
                x_tiles = []
                for t in range(NCHUNK):
                    x_t = xpool.tile([P, C], F32, tag="xt", name=f"xt{t}")
                    nc.sync.dma_start(x_t[:], x[_TS(t, P), :])
                    x_tiles.append(x_t)
                    if t == 3:
                        nc.sync.dma_start(
                            wqkv_sb[:],
                            wqkv.rearrange("(k p) m -> p k m", p=P))
                        nc.sync.dma_start(
                            qb_sb[:], qb.rearrange("(pr p) -> p pr", p=P))
                        nc.sync.dma_start(
                            kb_sb[:], kb.rearrange("(pr p) -> p pr", p=P))
                        nc.sync.dma_start(
                            vb_sb[:], vb.rearrange("(pr p) -> p pr", p=P))
                        nc.sync.dma_start(b2_sb[:], b2[None, :])
                        nc.gpsimd.dma_start(
                            xk[:], xown.rearrange("(h p) c -> p h c", p=P))
                        # b1 is 4096 tiny descriptors - keep it off the
                        # x-chunk queue (not needed until MLP)
                        nc.gpsimd.dma_start(
                            b1_sb[:], b1.rearrange("(k p) -> p k", p=P))

                def ln1_chunk(t):
                    x_t = x_tiles[t]
                    stats = ph1.tile([P, 2, 6], F32, tag="st")
                    nc.vector.bn_stats(stats[:, 0, :], x_t[:, 0:512])
                    nc.vector.bn_stats(stats[:, 1, :], x_t[:, 512:1024])
                    mv = ph1.tile([P, 2], F32, tag="mv")
                    nc.vector.bn_aggr(mv[:], stats[:])
                    # rstd = exp(-0.5*ln(var+eps)) - stays in the exp table
                    lnv = ph1.tile([P, 1], F32, tag="lnv")
                    nc.scalar.activation(lnv[:], mv[:, 1:2], AF.Ln,
                                         bias=eps_t[:], scale=1.0)
                    rstd = ph1.tile([P, 1], F32, tag="rs")
                    nc.scalar.activation(rstd[:], lnv[:], AF.Exp,
                                         bias=0.0, scale=-0.5)
                    nmr = ph1.tile([P, 1], F32, tag="nmr")
                    nc.vector.tensor_scalar(
                        out=nmr[:], in0=mv[:, 0:1], scalar1=rstd[:],
                        scalar2=-1.0, op0=OP.mult, op1=OP.mult)
                    xn = ph1.tile([P, C], BF16, tag="xn")
                    # normalize split ACT/DVE to balance engine load
                    nc.scalar.activation(xn[:, 0:512], x_t[:, 0:512],
                                         AF.Identity, bias=nmr[:],
                                         scale=rstd[:])
                    nc.vector.tensor_scalar(
                        out=xn[:, 512:1024], in0=x_t[:, 512:1024],
                        scalar1=mv[:, 0:1], scalar2=rstd[:],
                        op0=OP.subtract, op1=OP.mult)
                    for kg in range(2):
                        pt = pp_t1.tile([P, 4, P], BF16, tag="pt")
                        for k4 in range(4):
                            nc.tensor.transpose(
                                pt[:, k4, :], xn[:, _TS(kg * 4 + k4, P)],
                                id_b[:])
                        dst = ln1T[:, _TS(kg, 4), _TS(t, P)]
                        if kg == 0:
                            nc.scalar.activation(dst, pt[:], AF.Copy)
                        else:
                            nc.vector.tensor_copy(dst, pt[:])

                def qkv_block(nb, pr):
                    for w, (bias_sb, dst) in enumerate(
                            ((qb_sb, qT), (kb_sb, kT), (vb_sb, vT))):
                        ps = pp_q.tile([P, 512], F32, tag="ps")
                        off = w * C // 4 + pr * P
                        for kc in range(CCH):
                            nc.tensor.matmul(
                                ps[:],
                                wqkv_sb[:, kc, off:off + P],
                                ln1T[:, kc, _TS(nb, 512)],
                                start=(kc == 0), stop=(kc == CCH - 1))
                        nc.scalar.activation(
                            dst[:, pr, _TS(nb, 512)], ps[:], AF.Identity,
                            bias=bias_sb[:, pr:pr + 1], scale=1.0)

                # QKV for block nb emitted one chunk after its last LN
                # chunk so the ln1T evacuations have slack and the PE
                # QKV stream starts stall-free
                for t in range(NCHUNK):
                    ln1_chunk(t)
                    if t % 4 == 0 and t > 0:
                        qkv_block(t // 4 - 1, 0)
                        qkv_block(t // 4 - 1, 1)
                qkv_block(3, 0)
                qkv_block(3, 1)

            # ---------- phase 2: attention ----------------------------
            with (
                tc.tile_pool(name="ph3", bufs=2) as ph3,
                tc.tile_pool(name="pp_s", bufs=2, space="PSUM") as pp_s,
                tc.tile_pool(name="pp_o", bufs=1, space="PSUM") as pp_o,
                tc.tile_pool(name="pp_t", bufs=2, space="PSUM") as pp_t,
            ):
                pso = pp_o.tile([P, 2, 512], F32)

                def av_mm(h, p, expT, q, mc):
                    nc.tensor.matmul(
                        pso[0:HD + 1, q, :],
                        v_sb[:, h, mc, :],
                        expT[:, mc, _TS(q, 512)],
                        start=(mc == 0), stop=(mc == NCHUNK - 1))
                    if mc == NCHUNK - 1:
                        nc.vector.tensor_copy(
                            oT_sb[0:HD + 1, 2 * p + q, :],
                            pso[0:HD + 1, q, :])
                        if q == 1:
                            # this pass's denom rows -> [64,16] local DMA
                            # (pass p covers output rows 64p..64p+64)
                            nc.gpsimd.dma_start(
                                denT[64 * p:64 * p + 64, h, :],
                                oT_sb[HD:HD + 1, 2 * p:2 * p + 2, :]
                                .rearrange("o a (r j) -> o a r j",
                                           j=NCHUNK))
                            nc.vector.reciprocal(
                                rdenT[64 * p:64 * p + 64, h, :],
                                denT[64 * p:64 * p + 64, h, :])

                def sa_pass(h, p, av=None, fill=()):
                    # scores+exp for (h,p), interleaved 1:1 with the AV
                    # matmuls of the previous pass so ACT never starves,
                    # plus optional PE filler thunks (transposes).
                    pr, dp = h // 2, (h % 2) * HD
                    expT = ph3.tile([P, NCHUNK, 1024], BF16, tag="expT",
                                    name=f"expT{h}_{p}")
                    avl = ([(q, mc) for q in range(2)
                            for mc in range(NCHUNK)] if av else [])
                    fill = list(fill)
                    for mc in range(NCHUNK):
                        pss = pp_s.tile([P, 1024], F32, tag="ss",
                                        name=f"pss{h}_{p}_{mc}")
                        for q in range(2):
                            nc.tensor.matmul(
                                pss[:, _TS(q, 512)],
                                kT[dp:dp + HD, pr, _TS(mc, P)],
                                qT[dp:dp + HD, pr, _TS(2 * p + q, 512)],
                                start=True, stop=True)
                        nc.scalar.activation(
                            expT[:, mc, :], pss[:], AF.Exp, scale=SCALE)
                        for _ in range(2):
                            if avl:
                                q2, mc2 = avl.pop(0)
                                av_mm(av[0], av[1], av[2], q2, mc2)
                        if fill:
                            fill.pop(0)()
                    for q2, mc2 in avl:
                        av_mm(av[0], av[1], av[2], q2, mc2)
                    for f in fill:
                        f()
                    return expT

                def att_out(h, p, expT):
                    for q in range(2):
                        for mc in range(NCHUNK):
                            av_mm(h, p, expT, q, mc)

                oT_r = oT_sb[0:HD].rearrange("p a (r j) -> p j a r",
                                             j=NCHUNK)

                def fixup_thunks(h):
                    # x2[r, 64j+d] = oT[d, 16r+j] * rden[16r+j] + xown
                    def wave(w):
                        def go():
                            pt = pp_t.tile([P, 4, P], BF16, tag="tp",
                                           name=f"fx{h}_{w}")
                            for q in range(4):
                                j = w * 4 + q
                                nc.tensor.transpose(
                                    pt[:, q, 0:HD], oT_r[:, j, :, :],
                                    id_b[0:HD, 0:HD])
                            for q in range(4):
                                j = w * 4 + q
                                nc.vector.scalar_tensor_tensor(
                                    out=x2[:, h, _TS(j, HD)],
                                    in0=pt[:, q, 0:HD],
                                    scalar=rdenT[:, h, j:j + 1],
                                    in1=xk[:, h, _TS(j, HD)],
                                    op0=OP.mult, op1=OP.add)
                        return go
                    return [wave(w) for w in range(4)]

                def ln2_thunks(h):
                    def stats_go():
                        stats2 = ph3.tile([P, 2, 6], F32, tag="st2")
                        nc.vector.bn_stats(stats2[:, 0, :], x2[:, h, 0:512])
                        nc.vector.bn_stats(stats2[:, 1, :],
                                           x2[:, h, 512:1024])
                        mv2 = ph3.tile([P, 2], F32, tag="mv2")
                        nc.vector.bn_aggr(mv2[:], stats2[:])
                        lnv2 = ph3.tile([P, 1], F32, tag="lnv2")
                        nc.scalar.activation(lnv2[:], mv2[:, 1:2], AF.Ln,
                                             bias=eps_t[:], scale=1.0)
                        rstd2 = ph3.tile([P, 1], F32, tag="rs2")
                        nc.scalar.activation(rstd2[:], lnv2[:], AF.Exp,
                                             bias=0.0, scale=-0.5)
                        xn2 = ph3.tile([P, C], BF16, tag="xn2",
                                       name=f"xn2_{h}")
                        nc.vector.tensor_scalar(
                            out=xn2[:], in0=x2[:, h, :], scalar1=mv2[:, 0:1],
                            scalar2=rstd2[:], op0=OP.subtract, op1=OP.mult)
                        return xn2
                    box = {}

                    def tgo(kg):
                        def go():
                            if "xn2" not in box:
                                box["xn2"] = stats_go()
                            xn2 = box["xn2"]
                            pt = pp_t.tile([P, 4, P], BF16, tag="tp",
                                           name=f"l2{h}_{kg}")
                            for k4 in range(4):
                                nc.tensor.transpose(
                                    pt[:, k4, :],
                                    xn2[:, _TS(kg * 4 + k4, P)], id_b[:])
                            nc.vector.tensor_copy(
                                ln2T[:, _TS(kg, 4), _TS(h, P)], pt[:])
                        return go
                    return [tgo(0), tgo(1)]

                def v_nat_thunks(h):
                    pr, dp = h // 2, (h % 2) * HD

                    def grp(g):
                        def go():
                            pt = pp_t.tile([P, 4, P], BF16, tag="tp",
                                           name=f"vn{h}_{g}")
                            for q in range(4):
                                mc = g * 4 + q
                                nc.tensor.transpose(
                                    pt[:, q, 0:HD],
                                    vT[dp:dp + HD, pr, _TS(mc, P)],
                                    id_b[dp:dp + HD, dp:dp + HD])
                            nc.vector.tensor_copy(
                                v_sb[:, h, _TS(g, 4), 0:HD],
                                pt[:, :, 0:HD])
                        return go
                    return [grp(g) for g in range(4)]

                def w1_fetch(i):
                    wt = w1p.tile([P, CCH, 512], BF16, tag="w1t",
                                  name=f"w1t{i}")
                    nc.sync.dma_start(wt[:], w1r[:, :, _TS(i, 512)])
                    w1_tiles.append(wt)

                # -------- software pipeline --------
                for f in v_nat_thunks(0) + v_nat_thunks(1):
                    f()
                e00 = sa_pass(0, 0,
                              fill=v_nat_thunks(2) + v_nat_thunks(3))
                cur = sa_pass(0, 1, av=(0, 0, e00))
                w1_fetch(0)
                for h in range(1, NH):
                    e0 = sa_pass(h, 0, av=(h - 1, 1, cur))
                    cur = sa_pass(h, 1, av=(h, 0, e0),
                                  fill=fixup_thunks(h - 1)
                                  + ln2_thunks(h - 1))
                    w1_fetch(h)
                att_out(NH - 1, 1, cur)
                for f in fixup_thunks(NH - 1) + ln2_thunks(NH - 1):
                    f()

        # ---------------- phase 3: MLP --------------------------------
        with ExitStack() as mctx:
            mlp = mctx.enter_context(tc.tile_pool(name="mlp", bufs=1))
            w2p = mctx.enter_context(tc.tile_pool(name="w2p", bufs=4))
            h1T = mlp.tile([P, HKN, ROWS], BF16)
            w2_tiles = []

            def w2_fetch(i):
                w2t = w2p.tile([P, CCH, C], BF16, tag="w2t", name=f"w2t{i}")
                nc.gpsimd.dma_start(w2t[:], w2r[:, _TS(i, CCH), :])
                w2_tiles.append(w2t)

            for i in range(4):
                w2_fetch(i)
            ph5 = mctx.enter_context(tc.tile_pool(name="ph5", bufs=3))

            # MLP2 output accumulators keyed (j rowblock, cg colhalf).
            # Five ride interleaved inside MLP1 (psum: 3 banks MLP1 + 5
            # accumulators = 8); the last three sweep afterwards one at
            # a time with immediate evac+DMA so the exposed tail is a
            # single tile's worth of work.
            ACC0 = [(0, 0), (0, 1), (1, 0), (1, 1)]
            ACC1 = [(2, 0), (2, 1), (3, 0), (3, 1)]

            def acc_mm(tile, j, cg, hk):
                nc.tensor.matmul(
                    tile[:], h1T[:, hk, _TS(j, P)],
                    w2_tiles[hk // 8][:, hk % 8, _TS(cg, 512)],
                    start=False, stop=(hk == HKN - 1))

            def acc_evac(tile, j, cg):
                y_sb = ph5.tile([P, 512], F32, tag="ysb")
                nc.vector.tensor_add(y_sb[:], tile[:],
                                     x2[:, j, _TS(cg, 512)])
                nc.sync.dma_start(y[_TS(j, P), _TS(cg, 512)], y_sb[:])

            def psf_init(pool, accs, tag):
                tiles = {}
                for (j, cg) in accs:
                    t_ = pool.tile([P, 512], F32, tag=tag,
                                   bufs=len(accs), name=f"psf_{j}_{cg}")
                    nc.tensor.matmul(
                        t_[:], ones_row[:], b2_sb[0:1, _TS(cg, 512)],
                        start=True, stop=False)
                    tiles[(j, cg)] = t_
                return tiles

            # ACC0 lags 6 hk behind MLP1 so its first matmul never
            # head-of-line-blocks the PE on the w2 tile-0 DMA, then
            # catches up per hk.
            pf0 = mctx.enter_context(
                tc.tile_pool(name="pf0", bufs=1, space="PSUM"))
            psf0 = psf_init(pf0, ACC0, "f0")
            g0done = 0
            with tc.tile_pool(name="pp_m", bufs=3, space="PSUM") as pp_m:
                for hk in range(HKN):
                    if hk % 4 == 0 and 4 <= hk // 4 + 3 <= 7:
                        w1_fetch(hk // 4 + 3)
                    w1c = w1_tiles[hk // 4]
                    psh = pp_m.tile([P, ROWS], F32, tag="mm")
                    for kc in range(CCH):
                        nc.tensor.matmul(
                            psh[:], w1c[:, kc, _TS(hk % 4, P)],
                            ln2T[:, kc, :],
                            start=(kc == 0), stop=(kc == CCH - 1))
                    nc.scalar.activation(
                        h1T[:, hk, :], psh[:], AF.Relu,
                        bias=b1_sb[:, hk:hk + 1], scale=1.0)
                    if hk >= 6:
                        want = min(hk - 4, HKN)
                        while g0done < want:
                            for (j, cg) in ACC0:
                                acc_mm(psf0[(j, cg)], j, cg, g0done)
                            g0done += 1

            # pf1 opens in the banks pp_m just freed; ACC0's tail and
            # evacs overlap ACC1's accumulation
            with tc.tile_pool(name="pf1", bufs=1, space="PSUM") as pf1:
                psf1 = psf_init(pf1, ACC1, "f1")
                while g0done < HKN:
                    for (j, cg) in ACC0:
                        acc_mm(psf0[(j, cg)], j, cg, g0done)
                    g0done += 1
                for (j, cg) in ACC0:
                    acc_evac(psf0[(j, cg)], j, cg)
                for hk in range(HKN):
                    for (j, cg) in ACC1:
                        acc_mm(psf1[(j, cg)], j, cg, hk)
                for (j, cg) in ACC1:
                    acc_evac(psf1[(j, cg)], j, cg)


_NC_CACHE = {}
_TABLES_PATCHED = False


def _patch_act_tables():
    """Steer every activation we emit (ln/exp/relu/copy/identity) to the
    single covering table set so the kernel needs exactly one
    ACT_TABLE_LOAD (each load costs ~1.3us; the naive first-match
    selection reloads twice per LN because ln and exp default to
    different sets).  Order (= act_func_set_id) is preserved; only the
    compile-time selection changes, and the chosen set genuinely
    contains all five functions."""
    global _TABLES_PATCHED
    if _TABLES_PATCHED:
        return
    import concourse.hw_specs as hws
    import concourse.bacc as bacc_mod
    mine = {AF.Ln, AF.Exp, AF.Relu, AF.Copy, AF.Identity}
    orig = hws.get_activation_tables

    def patched(arch):
        tabs = orig(arch)
        cover = "natural_log_exp_and_others"
        if cover not in tabs or not mine <= tabs[cover]:
            return tabs
        return {name: (set(s) if name == cover else set(s) - mine)
                for name, s in tabs.items()}
    bacc_mod.get_activation_tables = patched
    _TABLES_PATCHED = True


def _get_nc():
    key = os.environ.get("KERNEL_REPS", "1")
    if key not in _NC_CACHE:
        _patch_act_tables()
        nc = bacc.Bacc("TRN2", target_bir_lowering=False, debug=False,
                       num_devices=8)
        _emit(nc)
        nc.compile()
        _NC_CACHE[key] = nc
    return _NC_CACHE[key]


def make_in_maps(x, qkv_w, qkv_b, w1, b1, w2, b2, ln1_g, ln1_b, ln2_g, ln2_b):
    import ml_dtypes
    x = np.asarray(x, dtype=np.float32)
    qkv_w = np.asarray(qkv_w, dtype=np.float32)
    qkv_b = np.asarray(qkv_b, dtype=np.float32)
    w1 = np.asarray(w1, dtype=np.float32)
    b1 = np.asarray(b1, dtype=np.float32)
    w2 = np.asarray(w2, dtype=np.float32)
    b2 = np.asarray(b2, dtype=np.float32)
    g1 = np.asarray(ln1_g, np.float32)
    be1 = np.asarray(ln1_b, np.float32)
    g2 = np.asarray(ln2_g, np.float32)
    be2 = np.asarray(ln2_b, np.float32)

    # fold LN affine params into the consuming weights (exact)
    qkv_w_f = qkv_w * g1[:, None]
    qkv_b_f = qkv_b + be1 @ qkv_w
    w1_f = np.ascontiguousarray(
        (w1 * g2[:, None]).astype(ml_dtypes.bfloat16))
    b1_f = b1 + be2 @ w1
    w2_b = np.ascontiguousarray(w2.astype(ml_dtypes.bfloat16))

    in_maps = []
    for core in range(8):
        b, g = divmod(core, 4)
        cs = slice(256 * g, 256 * (g + 1))
        wq = qkv_w_f[:, :C][:, cs]
        wk = qkv_w_f[:, C:2 * C][:, cs]
        wv = qkv_w_f[:, 2 * C:][:, cs]
        wqkv = np.ascontiguousarray(
            np.concatenate([wq, wk, wv], axis=1).astype(ml_dtypes.bfloat16))
        in_maps.append({
            "x": np.ascontiguousarray(x[b]),
            "xown": np.ascontiguousarray(x[b, 512 * g:512 * (g + 1)]),
            "wqkv": wqkv,
            "qb": np.ascontiguousarray(qkv_b_f[:C][cs]),
            "kb": np.ascontiguousarray(qkv_b_f[C:2 * C][cs]),
            "vb": np.ascontiguousarray(qkv_b_f[2 * C:][cs]),
            "w1": w1_f, "b1": b1_f, "w2": w2_b, "b2": b2,
        })
    return in_maps


LAST_RESULTS = None


def kernel(x, qkv_w, qkv_b, w1, b1, w2, b2, ln1_g, ln1_b, ln2_g, ln2_b):
    global LAST_RESULTS
    nc = _get_nc()
    in_maps = make_in_maps(x, qkv_w, qkv_b, w1, b1, w2, b2,
                           ln1_g, ln1_b, ln2_g, ln2_b)
    kwargs = {}
    if os.environ.get("KERNEL_TRACE"):
        kwargs = dict(trace=True, tmpdir=os.environ.get("KERNEL_TRACE_DIR"))
    res = run_bass_kernel_spmd(nc, in_maps, core_ids=list(range(8)), **kwargs)
    LAST_RESULTS = res
    out = np.empty((B, N, C), dtype=np.float32)
    for core in range(8):
        b, g = divmod(core, 4)
        out[b, 512 * g:512 * (g + 1)] = res.results[core]["y"]
    return out


# revision 34
# speedup vs baseline: 1.1917x; 1.1917x over previous
"""Trainium2 Bass kernel for a transformer block (dense_transformer).

Reference computation (B=2, N=2048, C=1024, 16 heads, head_dim=64):
    x = x + attn(LN1(x))            # attn WITHOUT output projection; heads
                                    # interleaved by the faithful reshape
    out = x + MLP(LN2(x))           # MLP = relu(x@w1+b1)@w2+b2

Sharding: 8 cores; core c handles batch b=c//4 and heads 4g..4g+3 (g=c%4).
Because the reference reshapes [B,H,N,hd]->[B,N,C] without transposing
heads back, head h's attention output occupies output rows
[128h, 128h+128) of batch b: out[b, 128h+r, 64m+d] = attn_h[16r+m, d].
So a (batch, 4-head) shard produces a contiguous [512, 1024] output slab
and the whole residual+MLP for those rows is local to the core.

v2 design (evidence-driven from the v1 trace):
  * LN gamma/beta folded into the consuming weights on the host
    (W' = g[:,None]*W, b' = b + beta@W) - exact; LN on chip is the pure
    (x-mu)*rsqrt(var+eps), with rsqrt = exp(-0.5*ln(var+eps)) so every
    ACT function used (ln/exp/relu/copy/identity) lives in ONE
    activation-table set -> zero table reloads.
  * x chunks DMA first; LN1(DVE) / transposes(PE) / QKV(PE) emitted
    interleaved per block so the PE stream never drains.
  * Attention is n-blocked in two 1024-wide passes per head; scoresT
    psum [128m,1024n] -> exp on ACT (sole exp engine = the attention
    bottleneck, so it does nothing else).  AV computed transposed
    (oT[d,n] = sum_m v[m,d] exp[m,n]) as 512-wide full-rate streams;
    ones column in v gives softmax denominators in oT row 64.  The
    interleaved output layout is restored by 16 stride-16 PE transposes
    per head; normalization and the residual add are fused into one DVE
    scalar_tensor_tensor per 64-col group.
  * MLP weights stream in big-contiguous-run tiles: w1 during
    attention, w2 racing MLP1/2 on the gpsimd DMA queue; prefetch issue
    points are placed after the previous slot-user's last read.
"""

import os
import sys
from contextlib import ExitStack

for _p in ("/opt/trn_rl_repo", "/root/.axon_site/_ro/trn_rl_repo"):
    if os.path.isdir(_p) and _p not in sys.path:
        sys.path.insert(0, _p)

import numpy as np

import concourse.bass as bass
import concourse.tile as tile
from concourse import bacc, mybir
from concourse.bass_utils import run_bass_kernel_spmd
from concourse.masks import make_identity

F32 = mybir.dt.float32
F32R = mybir.dt.float32r
BF16 = mybir.dt.bfloat16
AF = mybir.ActivationFunctionType
OP = mybir.AluOpType

P = 128
B, N, C = 2, 2048, 1024
H, HD = 16, 64
H4 = 4 * C
EPS = 1e-5
SCALE = 1.0 / 32.0  # 1/sqrt(C)

NH = 4            # heads per core
NPAIR = 2         # head pairs per core
ROWS = NH * P     # output rows per core (512)
NCHUNK = N // P   # 16 sequence chunks
CCH = C // P      # 8 channel chunks
HKN = H4 // P     # 32 hidden chunks

_TS = bass.ts


def _emit(nc):
    x = nc.dram_tensor("x", (N, C), F32, kind="ExternalInput").ap()
    xown = nc.dram_tensor("xown", (ROWS, C), F32, kind="ExternalInput").ap()
    wqkv = nc.dram_tensor("wqkv", (C, 3 * NH * HD), BF16,
                          kind="ExternalInput").ap()
    qb = nc.dram_tensor("qb", (NH * HD,), F32, kind="ExternalInput").ap()
    kb = nc.dram_tensor("kb", (NH * HD,), F32, kind="ExternalInput").ap()
    vb = nc.dram_tensor("vb", (NH * HD,), F32, kind="ExternalInput").ap()
    w1 = nc.dram_tensor("w1", (C, H4), BF16, kind="ExternalInput").ap()
    b1 = nc.dram_tensor("b1", (H4,), F32, kind="ExternalInput").ap()
    w2 = nc.dram_tensor("w2", (H4, C), BF16, kind="ExternalInput").ap()
    b2 = nc.dram_tensor("b2", (C,), F32R, kind="ExternalInput").ap()
    y = nc.dram_tensor("y", (ROWS, C), F32, kind="ExternalOutput").ap()

    reps = int(os.environ.get("KERNEL_REPS", "1"))
    with tile.TileContext(nc) as tc:
        for _ in range(reps):
            _body(tc, nc, x, xown, wqkv, qb, kb, vb, w1, b1, w2, b2, y)
    return nc


def _body(tc, nc, x, xown, wqkv, qb, kb, vb, w1, b1, w2, b2, y):
    w1r = w1.rearrange("(k p) hh -> p k hh", p=P)
    w2r = w2.rearrange("(k p) c -> p k c", p=P)
    with ExitStack() as ctx:
        singles = ctx.enter_context(tc.tile_pool(name="singles", bufs=1))
        w1p = ctx.enter_context(tc.tile_pool(name="w1p", bufs=4))
        w1_tiles = []

        id_b = singles.tile([P, P], BF16)
        make_identity(nc, id_b[:])
        eps_t = singles.tile([P, 1], F32)
        nc.vector.memset(eps_t[:], EPS)
        ones_f = singles.tile([1, P], F32)
        nc.vector.memset(ones_f[:], 1.0)
        ones_row = singles.tile([1, P], F32R)
        nc.vector.tensor_copy(ones_row[:], ones_f[:])
        b1_sb = singles.tile([P, HKN], F32)
        b2_sb = singles.tile([1, C], F32R)
        qb_sb = singles.tile([P, NPAIR], F32)
        kb_sb = singles.tile([P, NPAIR], F32)
        vb_sb = singles.tile([P, NPAIR], F32)

        x2 = singles.tile([P, NH, C], F32)
        ln2T = singles.tile([P, CCH, ROWS], BF16)

        with ExitStack() as actx:
            apool = actx.enter_context(tc.tile_pool(name="apool", bufs=1))
            xk = apool.tile([P, NH, C], F32)
            qT = apool.tile([P, NPAIR, N], BF16)
            kT = apool.tile([P, NPAIR, N], BF16)
            vT = apool.tile([P, NPAIR, N], BF16)
            v_sb = apool.tile([P, NH, NCHUNK, HD + 1], BF16)
            nc.vector.memset(v_sb[:, :, :, HD:HD + 1], 1.0)
            oT_sb = apool.tile([P, 4, 512], BF16)   # rows 0..64 used
            denT = apool.tile([P, NH, NCHUNK], F32)
            rdenT = apool.tile([P, NH, NCHUNK], F32)

            # ---------- phase 1: LN1 + transpose + QKV ----------------
            with (
                tc.tile_pool(name="ph1", bufs=3) as ph1,
                tc.tile_pool(name="ph1b", bufs=1) as ph1b,
                tc.tile_pool(name="xpool", bufs=6) as xpool,
                tc.tile_pool(name="pp_q", bufs=3, space="PSUM") as pp_q,
                tc.tile_pool(name="pp_t1", bufs=2, space="PSUM") as pp_t1,
            ):
                ln1T = ph1b.tile([P, CCH, N], BF16)
                wqkv_sb = ph1b.tile([P, CCH, 3 * NH * HD], BF16)
                x_tiles = []
                for t in range(NCHUNK):
                    x_t = xpool.tile([P, C], F32, tag="xt", name=f"xt{t}")
                    nc.sync.dma_start(x_t[:], x[_TS(t, P), :])
                    x_tiles.append(x_t)
                    if t == 3:
                        nc.sync.dma_start(
                            wqkv_sb[:],
                            wqkv.rearrange("(k p) m -> p k m", p=P))
                        nc.sync.dma_start(
                            qb_sb[:], qb.rearrange("(pr p) -> p pr", p=P))
                        nc.sync.dma_start(
                            kb_sb[:], kb.rearrange("(pr p) -> p pr", p=P))
                        nc.sync.dma_start(
                            vb_sb[:], vb.rearrange("(pr p) -> p pr", p=P))
                        nc.sync.dma_start(b2_sb[:], b2[None, :])
                        nc.gpsimd.dma_start(
                            xk[:], xown.rearrange("(h p) c -> p h c", p=P))
                        # b1 is 4096 tiny descriptors - keep it off the
                        # x-chunk queue (not needed until MLP)
                        nc.gpsimd.dma_start(
                            b1_sb[:], b1.rearrange("(k p) -> p k", p=P))

                def ln1_chunk(t):
                    x_t = x_tiles[t]
                    stats = ph1.tile([P, 2, 6], F32, tag="st")
                    nc.vector.bn_stats(stats[:, 0, :], x_t[:, 0:512])
                    nc.vector.bn_stats(stats[:, 1, :], x_t[:, 512:1024])
                    mv = ph1.tile([P, 2], F32, tag="mv")
                    nc.vector.bn_aggr(mv[:], stats[:])
                    # rstd = exp(-0.5*ln(var+eps)) - stays in the exp table
                    lnv = ph1.tile([P, 1], F32, tag="lnv")
                    nc.scalar.activation(lnv[:], mv[:, 1:2], AF.Ln,
                                         bias=eps_t[:], scale=1.0)
                    rstd = ph1.tile([P, 1], F32, tag="rs")
                    nc.scalar.activation(rstd[:], lnv[:], AF.Exp,
                                         bias=0.0, scale=-0.5)
                    nmr = ph1.tile([P, 1], F32, tag="nmr")
                    nc.vector.tensor_scalar(
                        out=nmr[:], in0=mv[:, 0:1], scalar1=rstd[:],
                        scalar2=-1.0, op0=OP.mult, op1=OP.mult)
                    xn = ph1.tile([P, C], BF16, tag="xn")
                    # normalize split ACT/DVE to balance engine load
                    nc.scalar.activation(xn[:, 0:512], x_t[:, 0:512],
                                         AF.Identity, bias=nmr[:],
                                         scale=rstd[:])
                    nc.vector.tensor_scalar(
                        out=xn[:, 512:1024], in0=x_t[:, 512:1024],
                        scalar1=mv[:, 0:1], scalar2=rstd[:],
                        op0=OP.subtract, op1=OP.mult)
                    for kg in range(2):
                        pt = pp_t1.tile([P, 4, P], BF16, tag="pt")
                        for k4 in range(4):
                            nc.tensor.transpose(
                                pt[:, k4, :], xn[:, _TS(kg * 4 + k4, P)],
                                id_b[:])
                        dst = ln1T[:, _TS(kg, 4), _TS(t, P)]
                        if kg == 0:
                            nc.scalar.activation(dst, pt[:], AF.Copy)
                        else:
                            nc.vector.tensor_copy(dst, pt[:])

                def qkv_block(nb, pr):
                    for w, (bias_sb, dst) in enumerate(
                            ((qb_sb, qT), (kb_sb, kT), (vb_sb, vT))):
                        ps = pp_q.tile([P, 512], F32, tag="ps")
                        off = w * C // 4 + pr * P
                        for kc in range(CCH):
                            nc.tensor.matmul(
                                ps[:],
                                wqkv_sb[:, kc, off:off + P],
                                ln1T[:, kc, _TS(nb, 512)],
                                start=(kc == 0), stop=(kc == CCH - 1))
                        nc.scalar.activation(
                            dst[:, pr, _TS(nb, 512)], ps[:], AF.Identity,
                            bias=bias_sb[:, pr:pr + 1], scale=1.0)

                # QKV for block nb emitted one chunk after its last LN
                # chunk so the ln1T evacuations have slack and the PE
                # QKV stream starts stall-free
                for t in range(NCHUNK):
                    ln1_chunk(t)
                    if t % 4 == 0 and t > 0:
                        qkv_block(t // 4 - 1, 0)
                        qkv_block(t // 4 - 1, 1)
                qkv_block(3, 0)
                qkv_block(3, 1)

            # ---------- phase 2: attention ----------------------------
            with (
                tc.tile_pool(name="ph3", bufs=2) as ph3,
                tc.tile_pool(name="pp_s", bufs=2, space="PSUM") as pp_s,
                tc.tile_pool(name="pp_o", bufs=1, space="PSUM") as pp_o,
                tc.tile_pool(name="pp_t", bufs=2, space="PSUM") as pp_t,
            ):
                pso = pp_o.tile([P, 2, 512], F32)

                def av_mm(h, p, expT, q, mc):
                    nc.tensor.matmul(
                        pso[0:HD + 1, q, :],
                        v_sb[:, h, mc, :],
                        expT[:, mc, _TS(q, 512)],
                        start=(mc == 0), stop=(mc == NCHUNK - 1))
                    if mc == NCHUNK - 1:
                        nc.vector.tensor_copy(
                            oT_sb[0:HD + 1, 2 * p + q, :],
                            pso[0:HD + 1, q, :])
                        if q == 1:
                            # this pass's denom rows -> [64,16] local DMA
                            # (pass p covers output rows 64p..64p+64)
                            nc.gpsimd.dma_start(
                                denT[64 * p:64 * p + 64, h, :],
                                oT_sb[HD:HD + 1, 2 * p:2 * p + 2, :]
                                .rearrange("o a (r j) -> o a r j",
                                           j=NCHUNK))
                            nc.vector.reciprocal(
                                rdenT[64 * p:64 * p + 64, h, :],
                                denT[64 * p:64 * p + 64, h, :])

                def sa_pass(h, p, av=None, fill=()):
                    # scores+exp for (h,p), interleaved 1:1 with the AV
                    # matmuls of the previous pass so ACT never starves,
                    # plus optional PE filler thunks (transposes).
                    pr, dp = h // 2, (h % 2) * HD
                    expT = ph3.tile([P, NCHUNK, 1024], BF16, tag="expT",
                                    name=f"expT{h}_{p}")
                    avl = ([(q, mc) for q in range(2)
                            for mc in range(NCHUNK)] if av else [])
                    fill = list(fill)
                    for mc in range(NCHUNK):
                        pss = pp_s.tile([P, 1024], F32, tag="ss",
                                        name=f"pss{h}_{p}_{mc}")
                        for q in range(2):
                            nc.tensor.matmul(
                                pss[:, _TS(q, 512)],
                                kT[dp:dp + HD, pr, _TS(mc, P)],
                                qT[dp:dp + HD, pr, _TS(2 * p + q, 512)],
                                start=True, stop=True)
                        nc.scalar.activation(
                            expT[:, mc, :], pss[:], AF.Exp, scale=SCALE)
                        for _ in range(2):
                            if avl:
                                q2, mc2 = avl.pop(0)
                                av_mm(av[0], av[1], av[2], q2, mc2)
                        if fill:
                            fill.pop(0)()
                    for q2, mc2 in avl:
                        av_mm(av[0], av[1], av[2], q2, mc2)
                    for f in fill:
                        f()
                    return expT

                def att_out(h, p, expT):
                    for q in range(2):
                        for mc in range(NCHUNK):
                            av_mm(h, p, expT, q, mc)

                oT_r = oT_sb[0:HD].rearrange("p a (r j) -> p j a r",
                                             j=NCHUNK)

                def fixup_thunks(h):
                    # x2[r, 64j+d] = oT[d, 16r+j] * rden[16r+j] + xown
                    def wave(w):
                        def go():
                            pt = pp_t.tile([P, 4, P], BF16, tag="tp",
                                           name=f"fx{h}_{w}")
                            for q in range(4):
                                j = w * 4 + q
                                nc.tensor.transpose(
                                    pt[:, q, 0:HD], oT_r[:, j, :, :],
                                    id_b[0:HD, 0:HD])
                            for q in range(4):
                                j = w * 4 + q
                                nc.vector.scalar_tensor_tensor(
                                    out=x2[:, h, _TS(j, HD)],
                                    in0=pt[:, q, 0:HD],
                                    scalar=rdenT[:, h, j:j + 1],
                                    in1=xk[:, h, _TS(j, HD)],
                                    op0=OP.mult, op1=OP.add)
                        return go
                    return [wave(w) for w in range(4)]

                def ln2_thunks(h):
                    def stats_go():
                        stats2 = ph3.tile([P, 2, 6], F32, tag="st2")
                        nc.vector.bn_stats(stats2[:, 0, :], x2[:, h, 0:512])
                        nc.vector.bn_stats(stats2[:, 1, :],
                                           x2[:, h, 512:1024])
                        mv2 = ph3.tile([P, 2], F32, tag="mv2")
                        nc.vector.bn_aggr(mv2[:], stats2[:])
                        lnv2 = ph3.tile([P, 1], F32, tag="lnv2")
                        nc.scalar.activation(lnv2[:], mv2[:, 1:2], AF.Ln,
                                             bias=eps_t[:], scale=1.0)
                        rstd2 = ph3.tile([P, 1], F32, tag="rs2")
                        nc.scalar.activation(rstd2[:], lnv2[:], AF.Exp,
                                             bias=0.0, scale=-0.5)
                        xn2 = ph3.tile([P, C], BF16, tag="xn2",
                                       name=f"xn2_{h}")
                        nc.vector.tensor_scalar(
                            out=xn2[:], in0=x2[:, h, :], scalar1=mv2[:, 0:1],
                            scalar2=rstd2[:], op0=OP.subtract, op1=OP.mult)
                        return xn2
                    box = {}

                    def tgo(kg):
                        def go():
                            if "xn2" not in box:
                                box["xn2"] = stats_go()
                            xn2 = box["xn2"]
                            pt = pp_t.tile([P, 4, P], BF16, tag="tp",
                                           name=f"l2{h}_{kg}")
                            for k4 in range(4):
                                nc.tensor.transpose(
                                    pt[:, k4, :],
                                    xn2[:, _TS(kg * 4 + k4, P)], id_b[:])
                            nc.vector.tensor_copy(
                                ln2T[:, _TS(kg, 4), _TS(h, P)], pt[:])
                        return go
                    return [tgo(0), tgo(1)]

                def v_nat_thunks(h):
                    pr, dp = h // 2, (h % 2) * HD

                    def grp(g):
                        def go():
                            pt = pp_t.tile([P, 4, P], BF16, tag="tp",
                                           name=f"vn{h}_{g}")
                            for q in range(4):
                                mc = g * 4 + q
                                nc.tensor.transpose(
                                    pt[:, q, 0:HD],
                                    vT[dp:dp + HD, pr, _TS(mc, P)],
                                    id_b[dp:dp + HD, dp:dp + HD])
                            nc.vector.tensor_copy(
                                v_sb[:, h, _TS(g, 4), 0:HD],
                                pt[:, :, 0:HD])
                        return go
                    return [grp(g) for g in range(4)]

                def w1_fetch(i):
                    wt = w1p.tile([P, CCH, 512], BF16, tag="w1t",
                                  name=f"w1t{i}")
                    nc.sync.dma_start(wt[:], w1r[:, :, _TS(i, 512)])
                    w1_tiles.append(wt)

                # -------- software pipeline --------
                for f in v_nat_thunks(0) + v_nat_thunks(1):
                    f()
                e00 = sa_pass(0, 0,
                              fill=v_nat_thunks(2) + v_nat_thunks(3))
                cur = sa_pass(0, 1, av=(0, 0, e00))
                w1_fetch(0)
                for h in range(1, NH):
                    e0 = sa_pass(h, 0, av=(h - 1, 1, cur))
                    cur = sa_pass(h, 1, av=(h, 0, e0),
                                  fill=fixup_thunks(h - 1)
                                  + ln2_thunks(h - 1))
                    w1_fetch(h)
                att_out(NH - 1, 1, cur)
                for f in fixup_thunks(NH - 1) + ln2_thunks(NH - 1):
                    f()

        # ---------------- phase 3: MLP --------------------------------
        with ExitStack() as mctx:
            mlp = mctx.enter_context(tc.tile_pool(name="mlp", bufs=1))
            w2p = mctx.enter_context(tc.tile_pool(name="w2p", bufs=4))
            h1T = mlp.tile([P, HKN, ROWS], BF16)
            w2_tiles = []

            def w2_fetch(i):
                w2t = w2p.tile([P, CCH, C], BF16, tag="w2t", name=f"w2t{i}")
                nc.gpsimd.dma_start(w2t[:], w2r[:, _TS(i, CCH), :])
                w2_tiles.append(w2t)

            for i in range(4):
                w2_fetch(i)
            ph5 = mctx.enter_context(tc.tile_pool(name="ph5", bufs=3))

            def mlp2_group(psfg, g, hk):
                # 4 matmuls of row-group g (rows 256g..256g+256) for hk
                for j2 in range(2):
                    j = 2 * g + j2
                    for cg in range(2):
                        nc.tensor.matmul(
                            psfg[j2 * 2 + cg][:],
                            h1T[:, hk, _TS(j, P)],
                            w2_tiles[hk // 8][:, hk % 8, _TS(cg, 512)],
                            start=False, stop=(hk == HKN - 1))

            def mlp2_evac(psfg, g):
                for j2 in range(2):
                    j = 2 * g + j2
                    for cg in range(2):
                        y_sb = ph5.tile([P, 512], F32, tag="ysb")
                        nc.vector.tensor_add(
                            y_sb[:], psfg[j2 * 2 + cg][:],
                            x2[:, j, _TS(cg, 512)])
                        nc.sync.dma_start(y[_TS(j, P), _TS(cg, 512)],
                                          y_sb[:])

            def psf_init(pool, g, tag):
                psfg = [pool.tile([P, 512], F32, tag=tag, bufs=4,
                                  name=f"psf{g}_{q}") for q in range(4)]
                for q in range(4):
                    nc.tensor.matmul(
                        psfg[q][:], ones_row[:],
                        b2_sb[0:1, _TS(q % 2, 512)],
                        start=True, stop=False)
                return psfg

            # MLP1 with row-group-0 of MLP2 riding behind it (psum: 3
            # banks MLP1 + 4 banks psf0 = 7).  g0 lags 6 hk so its first
            # matmul never head-of-line-blocks the PE on the w2 tile-0
            # DMA, then catches up two groups per hk.
            pf0 = mctx.enter_context(
                tc.tile_pool(name="pf0", bufs=1, space="PSUM"))
            psf0 = psf_init(pf0, 0, "f0")
            g0done = 0
            with tc.tile_pool(name="pp_m", bufs=3, space="PSUM") as pp_m:
                for hk in range(HKN):
                    if hk % 4 == 0 and 4 <= hk // 4 + 3 <= 7:
                        w1_fetch(hk // 4 + 3)
                    w1c = w1_tiles[hk // 4]
                    psh = pp_m.tile([P, ROWS], F32, tag="mm")
                    for kc in range(CCH):
                        nc.tensor.matmul(
                            psh[:], w1c[:, kc, _TS(hk % 4, P)],
                            ln2T[:, kc, :],
                            start=(kc == 0), stop=(kc == CCH - 1))
                    nc.scalar.activation(
                        h1T[:, hk, :], psh[:], AF.Relu,
                        bias=b1_sb[:, hk:hk + 1], scale=1.0)
                    if hk >= 6:
                        want = min(hk - 4, HKN)
                        while g0done < want:
                            mlp2_group(psf0, 0, g0done)
                            g0done += 1

            # pf1 opens in the banks pp_m just freed; g0's tail and evac
            # overlap g1's accumulation
            with tc.tile_pool(name="pf1", bufs=1, space="PSUM") as pf1:
                psf1 = psf_init(pf1, 1, "f1")
                while g0done < HKN:
                    mlp2_group(psf0, 0, g0done)
                    g0done += 1
                mlp2_evac(psf0, 0)
                for hk in range(HKN):
                    mlp2_group(psf1, 1, hk)
                mlp2_evac(psf1, 1)


_NC_CACHE = {}
_TABLES_PATCHED = False


def _patch_act_tables():
    """Steer every activation we emit (ln/exp/relu/copy/identity) to the
    single covering table set so the kernel needs exactly one
    ACT_TABLE_LOAD (each load costs ~1.3us; the naive first-match
    selection reloads twice per LN because ln and exp default to
    different sets).  Order (= act_func_set_id) is preserved; only the
    compile-time selection changes, and the chosen set genuinely
    contains all five functions."""
    global _TABLES_PATCHED
    if _TABLES_PATCHED:
        return
    import concourse.hw_specs as hws
    import concourse.bacc as bacc_mod
    mine = {AF.Ln, AF.Exp, AF.Relu, AF.Copy, AF.Identity}
    orig = hws.get_activation_tables

    def patched(arch):
        tabs = orig(arch)
        cover = "natural_log_exp_and_others"
        if cover not in tabs or not mine <= tabs[cover]:
            return tabs
        return {name: (set(s) if name == cover else set(s) - mine)
                for name, s in tabs.items()}
    bacc_mod.get_activation_tables = patched
    _TABLES_PATCHED = True


def _get_nc():
    key = os.environ.get("KERNEL_REPS", "1")
    if key not in _NC_CACHE:
        _patch_act_tables()
        nc = bacc.Bacc("TRN2", target_bir_lowering=False, debug=False,
                       num_devices=8)
        _emit(nc)
        nc.compile()
        _NC_CACHE[key] = nc
    return _NC_CACHE[key]


def make_in_maps(x, qkv_w, qkv_b, w1, b1, w2, b2, ln1_g, ln1_b, ln2_g, ln2_b):
    import ml_dtypes
    x = np.asarray(x, dtype=np.float32)
    qkv_w = np.asarray(qkv_w, dtype=np.float32)
    qkv_b = np.asarray(qkv_b, dtype=np.float32)
    w1 = np.asarray(w1, dtype=np.float32)
    b1 = np.asarray(b1, dtype=np.float32)
    w2 = np.asarray(w2, dtype=np.float32)
    b2 = np.asarray(b2, dtype=np.float32)
    g1 = np.asarray(ln1_g, np.float32)
    be1 = np.asarray(ln1_b, np.float32)
    g2 = np.asarray(ln2_g, np.float32)
    be2 = np.asarray(ln2_b, np.float32)

    # fold LN affine params into the consuming weights (exact)
    qkv_w_f = qkv_w * g1[:, None]
    qkv_b_f = qkv_b + be1 @ qkv_w
    w1_f = np.ascontiguousarray(
        (w1 * g2[:, None]).astype(ml_dtypes.bfloat16))
    b1_f = b1 + be2 @ w1
    w2_b = np.ascontiguousarray(w2.astype(ml_dtypes.bfloat16))

    in_maps = []
    for core in range(8):
        b, g = divmod(core, 4)
        cs = slice(256 * g, 256 * (g + 1))
        wq = qkv_w_f[:, :C][:, cs]
        wk = qkv_w_f[:, C:2 * C][:, cs]
        wv = qkv_w_f[:, 2 * C:][:, cs]
        wqkv = np.ascontiguousarray(
            np.concatenate([wq, wk, wv], axis=1).astype(ml_dtypes.bfloat16))
        in_maps.append({
            "x": np.ascontiguousarray(x[b]),
            "xown": np.ascontiguousarray(x[b, 512 * g:512 * (g + 1)]),
            "wqkv": wqkv,
            "qb": np.ascontiguousarray(qkv_b_f[:C][cs]),
            "kb": np.ascontiguousarray(qkv_b_f[C:2 * C][cs]),
            "vb": np.ascontiguousarray(qkv_b_f[2 * C:][cs]),
            "w1": w1_f, "b1": b1_f, "w2": w2_b, "b2": b2,
        })
    return in_maps


LAST_RESULTS = None


def kernel(x, qkv_w, qkv_b, w1, b1, w2, b2, ln1_g, ln1_b, ln2_g, ln2_b):
    global LAST_RESULTS
    nc = _get_nc()
    in_maps = make_in_maps(x, qkv_w, qkv_b, w1, b1, w2, b2,
                           ln1_g, ln1_b, ln2_g, ln2_b)
    kwargs = {}
    if os.environ.get("KERNEL_TRACE"):
        kwargs = dict(trace=True, tmpdir=os.environ.get("KERNEL_TRACE_DIR"))
    res = run_bass_kernel_spmd(nc, in_maps, core_ids=list(range(8)), **kwargs)
    LAST_RESULTS = res
    out = np.empty((B, N, C), dtype=np.float32)
    for core in range(8):
        b, g = divmod(core, 4)
        out[b, 512 * g:512 * (g + 1)] = res.results[core]["y"]
    return out
